# revision 38
# baseline (speedup 1.0000x reference)
"""Trainium2 Bass kernel for nn_MultiHeadSSAN: banded Q/K (prefix-sum windows
along feature_len) + multi-head self-attention, sharded over the feature_len
(L) axis across 8 NeuronCores.

v2 design (fp16 end-to-end data path, fp32 PSUM/scan accumulation):

  Band:  per (eb, ss) tile [128e, SB*CH] fp16, s-major l-inner.
         Products x*w split across GpSimd (4) and DVE (2); the fwd products
         are pre-shifted one step along l so a single inclusive
         tensor_tensor_scan yields the exclusive prefix directly (scan state
         is fp32 internally).  Assemble is 3 contiguous fp16 adds; the (s,l)
         -> (l,s) free-dim permute runs on the Scalar engine; stores go out
         as fp16 with 128B runs split across the Scalar and Sync queues.
         Chunk totals are extracted with small strided DVE copies and
         AllGathered per-eb (4 small collectives) so communication overlaps
         the remaining band compute.
  MHA:   per n: all matmuls fp16 (1 cyc/row, FWL weight loads).  Boundary
         constants fold into q/k via the PSUM-evacuation adds (no identity
         matmuls).  Softmax: (s,t) scores give -max (DVE reduce) and den
         (exp accum), lse = -(max+ln den) is split hi/lo fp16, PE-transposed
         to rows, and folded into the transposed (t,s) scores via rank-1
         PSUM accumulates, so the second exp emits normalized attn^T
         directly.  Output is stored fp16 and upcast on host.
"""
import math
import numpy as np

import concourse.bass as bass
import concourse.bacc as bacc
import concourse.mybir as mybir
import concourse.tile as tile
from concourse.bass_utils import run_bass_kernel_spmd

F32 = mybir.dt.float32
F16 = mybir.dt.float16
ALU = mybir.AluOpType
ACTF = mybir.ActivationFunctionType
AX = mybir.AxisListType


class Cfg:
    def __init__(self, S=256, L=512, E=512, H=4, NC=8, OFF=4, SB=64,
                 no_collective=False, reps=1,
                 skip_band=False, skip_attn=False, nmax=None, tune=None):
        self.S, self.L, self.E, self.H, self.NC = S, L, E, H, NC
        self.CH = L // NC              # L-chunk per core
        self.OFF = OFF                 # partner offset = n1 // CH
        assert OFF * 2 >= NC, "single-partner scheme needs OFF >= NC/2"
        self.n1 = self.n2 = OFF * self.CH
        self.HD = E // H
        assert self.HD == 128 and E % 128 == 0
        self.EB = E // 128             # e partition blocks
        self.SB = SB                   # band s-sub size
        assert S % SB == 0
        self.NSS = S // SB
        self.NST = (S + 127) // 128    # s tiles of <=128 in attention
        self.STW = min(128, S)
        assert self.STW == 128 and self.NST == 2
        self.no_collective = no_collective
        self.reps = reps
        self.skip_band = skip_band
        self.skip_attn = skip_attn
        self.nmax = nmax if nmax is not None else self.CH
        self.tune = dict(ps_a=3, ps_b=4, ps_t=1, dpool=3, scan=3, qkp=18, PT=18,
                         vp=10, osc=6, oo=4, lseflat=6)
        if tune: self.tune.update(tune)

    def key(self):
        return (self.S, self.L, self.E, self.H, self.NC, self.OFF, self.SB,
                self.no_collective, self.reps,
                self.skip_band, self.skip_attn, self.nmax,
                tuple(sorted(self.tune.items())))


def build_nc(cfg: Cfg) -> bass.Bass:
    S, L, E, H, NC = cfg.S, cfg.L, cfg.E, cfg.H, cfg.NC
    CH, EB, SB, HD = cfg.CH, cfg.EB, cfg.SB, cfg.HD
    NSS = cfg.NSS
    BW = SB * CH                       # band tile free width
    NST, STW = cfg.NST, cfg.STW
    NPAIR = 2

    nc = bacc.Bacc(None)
    # ---- parameters
    xband = nc.declare_dram_parameter("xband", [E, S, CH], F16, isOutput=False)
    xattn = nc.declare_dram_parameter("xattn", [E, CH, S], F16, isOutput=False)
    xp = nc.declare_dram_parameter("xp", [E, S, CH], F16, isOutput=False)
    wband = nc.declare_dram_parameter("wband", [6, E, CH], F16, isOutput=False)
    gate_in = nc.declare_dram_parameter("gate_in", [128, BW], F16, isOutput=False)
    bqin = nc.declare_dram_parameter("bqin", [E, S], F16, isOutput=False)
    bkin = nc.declare_dram_parameter("bkin", [E, S], F16, isOutput=False)
    wq = nc.declare_dram_parameter("wq", [E, E], F16, isOutput=False)
    wk = nc.declare_dram_parameter("wk", [E, E], F16, isOutput=False)
    wv = nc.declare_dram_parameter("wv", [E, E], F16, isOutput=False)
    wo = nc.declare_dram_parameter("wo", [E, E], F16, isOutput=False)
    biasv = nc.declare_dram_parameter("biasv", [1, E], F16, isOutput=False)
    biasc = nc.declare_dram_parameter("biasc", [E, 4], F32, isOutput=False)
    ident_in = nc.declare_dram_parameter("ident_in", [128, 128], F16, isOutput=False)
    out = nc.declare_dram_parameter("out", [E, CH, S], F16, isOutput=True)

    # ---- internal DRAM
    qdram = nc.dram_tensor("qdram", [E, CH, S], F16)
    kdram = nc.dram_tensor("kdram", [E, CH, S], F16)

    with tile.TileContext(nc) as tc:
        with (
            tc.tile_pool(name="const", bufs=1) as cpool,
            tc.tile_pool(name="ps_a", bufs=cfg.tune["ps_a"], space="PSUM") as ps_a,
            tc.tile_pool(name="ps_b", bufs=cfg.tune["ps_b"], space="PSUM") as ps_b,
            tc.tile_pool(name="ps_t", bufs=cfg.tune["ps_t"], space="PSUM") as ps_t,
        ):
            # ================= constants =================
            gate = cpool.tile([128, BW], F16, name="gate")
            nc.sync.dma_start(gate[:], gate_in[:, :])
            ident = cpool.tile([128, 128], F16, name="ident")
            nc.sync.dma_start(ident[:], ident_in[:, :])

            biasv_sb = cpool.tile([1, E], F16, name="biasv_sb")
            nc.sync.dma_start(biasv_sb[:], biasv[:, :])
            biasc_sb = cpool.tile([128, 4 * EB], F32, name="biasc_sb")
            for eb in range(EB):
                nc.sync.dma_start(biasc_sb[:, 4 * eb:4 * (eb + 1)],
                                  biasc[eb * 128:(eb + 1) * 128, :])
            ones_row = cpool.tile([1, max(S, 128)], F16, name="ones_row")
            nc.vector.memset(ones_row[:], 1.0)
            ones2 = cpool.tile([2, 128], F16, name="ones2")
            nc.vector.memset(ones2[:], 1.0)

            wband_sb = []
            for kind in range(6):
                row = []
                for eb in range(EB):
                    t = cpool.tile([128, CH], F16, name=f"wband_{kind}_{eb}")
                    nc.sync.dma_start(t[:], wband[kind, eb * 128:(eb + 1) * 128, :])
                    row.append(t)
                wband_sb.append(row)

            def load_w(dram, nm):
                tiles = []
                for eb in range(EB):
                    t = cpool.tile([128, E], F16, name=f"{nm}_{eb}")
                    nc.sync.dma_start(t[:], dram[eb * 128:(eb + 1) * 128, :])
                    tiles.append(t)
                return tiles

            wq_sb = load_w(wq, "wq")
            wk_sb = load_w(wk, "wk")
            wv_sb = load_w(wv, "wv")
            wo_sb = load_w(wo, "wo")

            # host-computed boundary terms per eb
            bq_eb, bk_eb = [], []
            for eb in range(EB):
                er = slice(eb * 128, (eb + 1) * 128)
                t = cpool.tile([128, S], F16, name=f"bq_{eb}")
                nc.sync.dma_start(t[:], bqin[er, :])
                bq_eb.append(t)
                t = cpool.tile([128, S], F16, name=f"bk_{eb}")
                nc.sync.dma_start(t[:], bkin[er, :])
                bk_eb.append(t)

            def emit_band():
                with (
                    tc.tile_pool(name="bin", bufs=2) as binp,
                    tc.tile_pool(name="prod", bufs=5) as ppool,
                    tc.tile_pool(name="scan", bufs=cfg.tune["scan"]) as spool,
                    tc.tile_pool(name="asm", bufs=2) as apool,
                ):
                    for eb in range(EB):
                        er = slice(eb * 128, (eb + 1) * 128)
                        for ss in range(NSS):
                            sr = slice(ss * SB, (ss + 1) * SB)
                            xb = binp.tile([128, BW], F16, name="xb", tag="xb")
                            nc.sync.dma_start(xb[:], xband[er, sr, :])
                            xpb = binp.tile([128, BW], F16, name="xpb", tag="xpb")
                            nc.sync.dma_start(xpb[:], xp[er, sr, :])
                            x3 = xb[:].rearrange("p (s l) -> p s l", l=CH)
                            xp3 = xpb[:].rearrange("p (s l) -> p s l", l=CH)

                            def prod(kind, src3, nm, eng):
                                p = ppool.tile([128, BW], F16, name=nm,
                                               tag=f"prod_{eng}",
                                               bufs=(4 if eng == "g" else 2))
                                p3 = p[:].rearrange("p (s l) -> p s l", l=CH)
                                e = nc.gpsimd if eng == "g" else nc.vector
                                wb = wband_sb[kind][eb][:].unsqueeze(1) \
                                    .broadcast_to([128, SB, CH])
                                e.tensor_tensor(p3, src3, wb, op=ALU.mult)
                                return p

                            def scan(p, nm):
                                o = spool.tile([128, BW], F16, name=nm, tag="scan")
                                nc.vector.tensor_tensor_scan(
                                    o[:], gate[:], p[:], 0.0,
                                    op0=ALU.mult, op1=ALU.add)
                                return o

                            def half(qk, kf, ks, kp_, dram, store_eng):
                                # kf: fwd kind, ks: bwd kind, kp_: partner
                                peng = "v" if qk == "q" else "g"
                                pf = prod(kf, x3, "pf", "g")
                                ps_ = prod(ks, x3, "ps", "g")
                                pp = prod(kp_, xp3, "pp", peng)
                                # combined scan of (shift(pf) + pp - ps); the
                                # fwd shift is applied while combining
                                c1 = apool.tile([128, BW], F16, name=f"c1{qk}",
                                                tag="ts")
                                c13 = c1[:].rearrange("p (s l) -> p s l", l=CH)
                                pf3 = pf[:].rearrange("p (s l) -> p s l", l=CH)
                                pp3 = pp[:].rearrange("p (s l) -> p s l", l=CH)
                                nc.vector.tensor_tensor(
                                    c13[:, :, 1:CH], pf3[:, :, 0:CH - 1],
                                    pp3[:, :, 1:CH], op=ALU.add)
                                nc.vector.tensor_copy(c13[:, :, 0:1],
                                                      pp3[:, :, 0:1])
                                c2 = apool.tile([128, BW], F16, name=f"c2{qk}",
                                                tag="t1")
                                nc.vector.tensor_tensor(c2[:], c1[:], ps_[:],
                                                        op=ALU.subtract)
                                I = scan(c2, "I")
                                t2 = apool.tile([128, BW], F16, name=f"t2{qk}",
                                                tag="ts")
                                nc.vector.tensor_tensor(t2[:], xb[:], I[:],
                                                        op=ALU.add)
                                # free-dim permute (s,l)->(l,s) on Scalar
                                o2 = apool.tile([128, BW], F16, name=f"o2{qk}",
                                                tag="o2")
                                nc.scalar.copy(
                                    o2[:].rearrange("p (l s) -> p l s", s=SB),
                                    t2[:].rearrange("p (s l) -> p l s", l=CH))
                                store_eng.dma_start(
                                    dram[er, 0:CH, sr],
                                    o2[:].rearrange("p (l s) -> p l s", s=SB))

                            half("q", 0, 2, 4, qdram, nc.scalar)
                            half("k", 1, 3, 5, kdram, nc.sync)


            # ================= B-terms =================
            def emit_b():
                Bqp, Bkp = [], []  # fp16 [128, S] per fm, proj-space
                for qk, w_sb, B_eb, bj, dst in (
                        ("q", wq_sb, bq_eb, 0, Bqp),
                        ("k", wk_sb, bk_eb, 1, Bkp)):
                    for fm in range(EB):
                        fr = slice(fm * 128, (fm + 1) * 128)
                        acc = ps_a.tile([128, S], F32, name=f"psB{qk}{fm}",
                                        tag="ps_mm")
                        for eb in range(EB):
                            nc.tensor.matmul(acc[:], w_sb[eb][:, fr],
                                             B_eb[eb][:],
                                             start=(eb == 0), stop=(eb == EB - 1))
                        o = cpool.tile([128, S], F16, name=f"B{qk}p_{fm}")
                        nc.vector.tensor_scalar_add(
                            o[:], acc[:],
                            biasc_sb[:, 4 * fm + bj:4 * fm + bj + 1])
                        dst.append(o)
                return Bqp, Bkp

            def attn_stage1(n0, qt2, kt2, xt2, Bqp, Bkp, apool):
                """proj + v-proj + pass1 softmax stats + lse -> state dict."""
                T = cfg.tune
                NP = NPAIR

                def proj(w_sb, src2, Bp, nm):
                    outt = []
                    for fm in range(EB):
                        fr = slice(fm * 128, (fm + 1) * 128)
                        acc = ps_a.tile([128, NP * S], F32, name=f"ps{nm}{fm}",
                                        tag="ps_mm")
                        for eb in range(EB):
                            nc.tensor.matmul(acc[:], w_sb[eb][:, fr],
                                             src2[eb][:],
                                             start=(eb == 0), stop=(eb == EB - 1))
                        o = apool.tile([128, NP * S], F16, name=f"{nm}_{fm}",
                                       tag="qkp", bufs=T["qkp"])
                        for j in range(NP):
                            js = slice(j * S, (j + 1) * S)
                            nc.vector.tensor_tensor(o[:, js], acc[:, js],
                                                    Bp[fm][:], op=ALU.add)
                        outt.append(o)
                    return outt

                qp = proj(wq_sb, qt2, Bqp, "qp")
                kp = proj(wk_sb, kt2, Bkp, "kp")

                def hv(p, h, j):
                    return p[h][:, j * S:(j + 1) * S]

                vp = [[None] * NST for _ in range(NP)]
                for j in range(NP):
                    for st in range(NST):
                        scols = slice(j * S + st * 128, j * S + st * 128 + STW)
                        acc = ps_a.tile([STW, E], F32, name=f"psv{j}{st}",
                                        tag="ps_mm")
                        for eb in range(EB):
                            nc.tensor.matmul(acc[:], xt2[eb][:, scols],
                                             wv_sb[eb][:],
                                             start=(eb == 0), stop=False)
                        nc.tensor.matmul(acc[:], ones_row[:1, :STW],
                                         biasv_sb[:1, :], start=False, stop=True)
                        o = apool.tile([STW, E], F16, name=f"vp{j}{st}",
                                       tag="vp", bufs=T["vp"])
                        nc.vector.tensor_copy(o[:], acc[:])
                        vp[j][st] = o

                # pass 1: (s,t) scores -> -max, den (both j in one den tile)
                nmax_c = []
                den_pair = apool.tile([STW, NP * 2 * H], F32, name="denp",
                                      tag="denp", bufs=3)
                escr = apool.tile([STW, S], F16, name="escr", tag="escr", bufs=2)
                for j in range(NP):
                    nm_ = apool.tile([STW, 2 * H], F32, name=f"nmaxc{j}",
                                     tag="nmaxc", bufs=4)
                    nmax_c.append(nm_)
                    for st in range(NST):
                        scols = slice(st * 128, st * 128 + STW)
                        for h in range(H):
                            c = h * NST + st
                            accs = ps_b.tile([STW, S], F32, name=f"ps1{j}{st}{h}",
                                             tag="ps_sc")
                            nc.tensor.matmul(accs[:], hv(qp, h, j)[:, scols],
                                             hv(kp, h, j), start=True, stop=True)
                            nc.vector.tensor_reduce(
                                nm_[:, c:c + 1], accs[:], axis=AX.X,
                                op=ALU.max, negate=True)
                            nc.scalar.activation(
                                escr[:], accs[:], ACTF.Exp,
                                bias=nm_[:, c:c + 1], scale=1.0,
                                accum_out=den_pair[:, j * 2 * H + c:j * 2 * H + c + 1])
                # ONE Ln for the whole pair
                ln_pair = apool.tile([STW, NP * 2 * H], F32, name="lnp",
                                     tag="lnp", bufs=3)
                nc.scalar.activation(ln_pair[:], den_pair[:], ACTF.Ln)
                lseflat = []
                for j in range(NP):
                    lse32 = apool.tile([STW, 2 * H], F32, name=f"lse32{j}",
                                       tag="lse32", bufs=4)
                    nc.vector.tensor_tensor(
                        lse32[:], nmax_c[j][:],
                        ln_pair[:, j * 2 * H:(j + 1) * 2 * H],
                        op=ALU.subtract)  # -(max+ln den)
                    pk = apool.tile([STW, 4 * H], F16, name=f"lsepack{j}",
                                    tag="lsepack", bufs=4)
                    nc.vector.tensor_copy(pk[:, 0:2 * H], lse32[:])
                    resid = apool.tile([STW, 2 * H], F32, name=f"resid{j}",
                                       tag="resid", bufs=4)
                    nc.vector.tensor_tensor(resid[:], lse32[:], pk[:, 0:2 * H],
                                            op=ALU.subtract)
                    nc.vector.tensor_copy(pk[:, 2 * H:4 * H], resid[:])
                    lf = apool.tile([2, STW * 2 * H], F16, name=f"lseflat{j}",
                                    tag="lseflat", bufs=T["lseflat"])
                    for hl in range(2):
                        nc.sync.dma_start(
                            lf[hl:hl + 1].rearrange("o (s r) -> o s r",
                                                    r=2 * H),
                            pk[:, hl * 2 * H:(hl + 1) * 2 * H])
                    lseflat.append(lf)
                return dict(n0=n0, qp=qp, kp=kp, vp=vp, lseflat=lseflat, hv=hv)

            def attn_stage2(stt, apool):
                """pass2 + attn@V + out-projection for a stage1'd pair."""
                T = cfg.tune
                NP = NPAIR
                n0, qp, kp, vp, lseflat, hv = (stt["n0"], stt["qp"], stt["kp"],
                                               stt["vp"], stt["lseflat"],
                                               stt["hv"])
                PT = [[[None] * NST for _ in range(H)] for _ in range(NP)]
                for j in range(NP):
                    lse_rs = lseflat[j][:].rearrange("p (s r) -> p r s",
                                                     r=2 * H)
                    for grp in range(2):
                        accs2 = []
                        for h2 in range(2):
                            h = grp * 2 + h2
                            for tt in range(NST):
                                tcols = slice(tt * 128, tt * 128 + STW)
                                acc = ps_b.tile([STW, S], F32,
                                                name=f"ps2{j}{h}{tt}",
                                                tag="ps_sc")
                                nc.tensor.matmul(acc[:], hv(kp, h, j)[:, tcols],
                                                 hv(qp, h, j),
                                                 start=True, stop=False)
                                accs2.append((acc, h, tt))
                        for acc, h, tt in accs2:
                            r0 = h * NST
                            nc.tensor.matmul(
                                acc[:], ones2[:2, :STW],
                                lse_rs[:, r0:r0 + NST, :],
                                start=False, stop=True)
                            p = apool.tile([STW, S], F16, name=f"PT{j}{h}{tt}",
                                           tag="PT", bufs=T["PT"])
                            nc.scalar.activation(p[:], acc[:], ACTF.Exp)
                            PT[j][h][tt] = p

                osc = []
                for h in range(H):
                    hr = slice(h * HD, (h + 1) * HD)
                    acc = ps_t.tile([HD, NP * S], F32, name=f"pso{h}",
                                    tag="ps_oo")
                    for j in range(NP):
                        js = slice(j * S, (j + 1) * S)
                        for tt in range(NST):
                            nc.tensor.matmul(acc[:, js], vp[j][tt][:, hr],
                                             PT[j][h][tt][:],
                                             start=(tt == 0), stop=(tt == NST - 1))
                    o = apool.tile([HD, NP * S], F16, name=f"osc{h}", tag="osc",
                                   bufs=T["osc"])
                    nc.vector.tensor_copy(o[:], acc[:])
                    osc.append(o)

                for gm in range(EB):
                    gr = slice(gm * 128, (gm + 1) * 128)
                    acc = ps_a.tile([128, NP * S], F32, name=f"psout{gm}",
                                    tag="ps_mm")
                    for fm in range(EB):
                        nc.tensor.matmul(acc[:], wo_sb[fm][:, gr], osc[fm][:],
                                         start=(fm == 0), stop=(fm == EB - 1))
                    o = apool.tile([128, NP * S], F16, name=f"oo{gm}", tag="oo",
                                   bufs=T["oo"])
                    nc.vector.tensor_scalar_add(
                        o[:], acc[:], biasc_sb[:, 4 * gm + 3:4 * gm + 4])
                    nc.sync.dma_start(
                        out[gr, n0:n0 + NP, :],
                        o[:].rearrange("p (j s) -> p j s", j=NP))

            def emit_attn_all(Bqp, Bkp):
                with (
                    tc.tile_pool(name="dpool", bufs=cfg.tune["dpool"]) as dpool,
                    tc.tile_pool(name="attn", bufs=2) as apool,
                ):
                    NMAX = cfg.nmax if not cfg.skip_attn else 0
                    assert NMAX % NPAIR == 0
                    prev = None
                    for n0 in range(0, NMAX, NPAIR):
                        qt2, kt2, xt2 = [], [], []
                        nsl = slice(n0, n0 + NPAIR)
                        for eb in range(EB):
                            er = slice(eb * 128, (eb + 1) * 128)
                            t = dpool.tile([128, NPAIR * S], F16, name=f"qt{eb}",
                                           tag=f"qt{eb}")
                            nc.sync.dma_start(t[:], qdram[er, nsl, :])
                            qt2.append(t)
                            t = dpool.tile([128, NPAIR * S], F16, name=f"kt{eb}",
                                           tag=f"kt{eb}")
                            nc.sync.dma_start(t[:], kdram[er, nsl, :])
                            kt2.append(t)
                            t = dpool.tile([128, NPAIR * S], F16, name=f"xt{eb}",
                                           tag=f"xt{eb}")
                            nc.sync.dma_start(t[:], xattn[er, nsl, :])
                            xt2.append(t)
                        cur = attn_stage1(n0, qt2, kt2, xt2, Bqp, Bkp, apool)
                        if prev is not None:
                            attn_stage2(prev, apool)
                        prev = cur
                    if prev is not None:
                        attn_stage2(prev, apool)

            for _rep in range(cfg.reps):
                Bqp, Bkp = emit_b()
                if not cfg.skip_band:
                    emit_band()
                emit_attn_all(Bqp, Bkp)

    nc.finalize()
    return nc


# ============================================================
# host side
# ============================================================

def prep_inputs(cfg: Cfg, x, a, b, c, d, in_proj_w, in_proj_b, out_w, out_b):
    S, L, E, NC, CH, OFF = cfg.S, cfg.L, cfg.E, cfg.NC, cfg.CH, cfg.OFF
    f32, f16 = np.float32, np.float16
    x = np.asarray(x, f32)
    xg = np.ascontiguousarray(x.transpose(2, 0, 1))     # (E, S, L)
    hd = cfg.HD
    scl = 1.0 / math.sqrt(hd)
    wq = np.ascontiguousarray(in_proj_w[:E].T * scl).astype(f16)
    wk = np.ascontiguousarray(in_proj_w[E:2 * E].T).astype(f16)
    wv = np.ascontiguousarray(in_proj_w[2 * E:].T).astype(f16)
    wo = np.ascontiguousarray(out_w.T).astype(f16)
    bq = in_proj_b[:E] * scl
    bk = in_proj_b[E:2 * E]
    bv = in_proj_b[2 * E:]
    bo = out_b
    biasv = np.asarray(bv, f16).reshape(1, E)
    # last-column fwd weights (per core below)
    biasc = np.ascontiguousarray(
        np.stack([bq, bk, bv, bo]).astype(f32).T)       # (E, 4)
    ident = np.eye(128, dtype=f16)

    gate = np.ones((128, cfg.SB * CH), f16)
    gate[:, ::CH] = 0.0

    # boundary chunk totals: T[kind][j][e,s] = sum_{l in chunk j} x[s,l,e]*w[l,e]
    xr = x.reshape(S, NC, CH, E)
    Tt = {}
    for nmw, w in (("a", a), ("b", b), ("c", c), ("d", d)):
        Tt[nmw] = np.einsum("sjle,jle->jes", xr,
                            np.asarray(w, f32).reshape(NC, CH, E),
                            optimize=True)

    in_maps = []
    for k in range(NC):
        chk = slice(CH * k, CH * (k + 1))
        xbandc = np.ascontiguousarray(xg[:, :, chk]).astype(f16)
        xattnc = np.ascontiguousarray(
            xg[:, :, chk].transpose(0, 2, 1)).astype(f16)
        if k >= OFF:
            pf = slice(CH * (k - OFF), CH * (k - OFF + 1))
            xpc = np.ascontiguousarray(xg[:, :, pf]).astype(f16)
            w1 = -a[pf].astype(f32)
            w2 = -b[pf].astype(f32)
        else:
            st = CH * (k + OFF) - 1
            xpc = np.zeros((E, S, CH), f16)
            xpc[:, :, 1:] = xg[:, :, st + 1:st + CH]
            w1 = np.zeros((CH, E), f32)
            w1[1:] = c[st + 1:st + CH]
            w2 = np.zeros((CH, E), f32)
            w2[1:] = d[st + 1:st + CH]
        wbandc = np.ascontiguousarray(
            np.stack([a[chk], b[chk], c[chk], d[chk], w1, w2])
            .transpose(0, 2, 1)).astype(f16)            # (6, E, CH)
        jA = slice(max(0, k - OFF), k)
        jC = slice(k, min(k + OFF - 1, NC - 1) + 1)
        bqc = (Tt["a"][jA].sum(0) + Tt["c"][jC].sum(0)).astype(f16)
        bkc = (Tt["b"][jA].sum(0) + Tt["d"][jC].sum(0)).astype(f16)
        in_maps.append(dict(
            xband=xbandc, xattn=xattnc, xp=xpc,
            wband=wbandc, gate_in=gate, bqin=bqc, bkin=bkc,
            wq=wq, wk=wk, wv=wv, wo=wo, biasv=biasv, biasc=biasc,
            ident_in=ident,
        ))
    return in_maps


_CACHE = {}


def run(cfg: Cfg, inputs, core_ids=None, **kw):
    key = cfg.key()
    if key not in _CACHE:
        _CACHE[key] = build_nc(cfg)
    nc = _CACHE[key]
    in_maps = prep_inputs(
        cfg, inputs["x"], inputs["a"], inputs["b"], inputs["c"], inputs["d"],
        inputs["in_proj_w"], inputs["in_proj_b"], inputs["out_w"], inputs["out_b"])
    res = run_bass_kernel_spmd(nc, in_maps, core_ids or list(range(cfg.NC)), **kw)
    S, L, E, CH = cfg.S, cfg.L, cfg.E, cfg.CH
    full = np.empty((S, L, E), np.float32)
    for k in range(cfg.NC):
        # out is (E, CH, S) fp16
        full[:, CH * k:CH * (k + 1), :] = \
            res.results[k]["out"].astype(np.float32).transpose(2, 1, 0)
    return full, res


def kernel(**inputs) -> np.ndarray:
    assert int(inputs["n1"]) == 256 and int(inputs["n2"]) == 256
    cfg = Cfg()
    out, _ = run(cfg, inputs)
    return out


# revision 39
# speedup vs baseline: 1.0965x; 1.0965x over previous
"""Trainium2 Bass kernel for nn_MultiHeadSSAN: banded Q/K (prefix-sum windows
along feature_len) + multi-head self-attention, sharded over the feature_len
(L) axis across 8 NeuronCores.

v2 design (fp16 end-to-end data path, fp32 PSUM/scan accumulation):

  Band:  per (eb, ss) tile [128e, SB*CH] fp16, s-major l-inner.
         Products x*w split across GpSimd (4) and DVE (2); the fwd products
         are pre-shifted one step along l so a single inclusive
         tensor_tensor_scan yields the exclusive prefix directly (scan state
         is fp32 internally).  Assemble is 3 contiguous fp16 adds; the (s,l)
         -> (l,s) free-dim permute runs on the Scalar engine; stores go out
         as fp16 with 128B runs split across the Scalar and Sync queues.
         Chunk totals are extracted with small strided DVE copies and
         AllGathered per-eb (4 small collectives) so communication overlaps
         the remaining band compute.
  MHA:   per n: all matmuls fp16 (1 cyc/row, FWL weight loads).  Boundary
         constants fold into q/k via the PSUM-evacuation adds (no identity
         matmuls).  Softmax: (s,t) scores give -max (DVE reduce) and den
         (exp accum), lse = -(max+ln den) is split hi/lo fp16, PE-transposed
         to rows, and folded into the transposed (t,s) scores via rank-1
         PSUM accumulates, so the second exp emits normalized attn^T
         directly.  Output is stored fp16 and upcast on host.
"""
import math
import numpy as np

import concourse.bass as bass
import concourse.bacc as bacc
import concourse.mybir as mybir
import concourse.tile as tile
from concourse.bass_utils import run_bass_kernel_spmd

F32 = mybir.dt.float32
F16 = mybir.dt.float16
ALU = mybir.AluOpType
ACTF = mybir.ActivationFunctionType
AX = mybir.AxisListType


class Cfg:
    def __init__(self, S=256, L=512, E=512, H=4, NC=8, OFF=4, SB=64,
                 no_collective=False, reps=1,
                 skip_band=False, skip_attn=False, nmax=None, tune=None):
        self.S, self.L, self.E, self.H, self.NC = S, L, E, H, NC
        self.CH = L // NC              # L-chunk per core
        self.OFF = OFF                 # partner offset = n1 // CH
        assert OFF * 2 >= NC, "single-partner scheme needs OFF >= NC/2"
        self.n1 = self.n2 = OFF * self.CH
        self.HD = E // H
        assert self.HD == 128 and E % 128 == 0
        self.EB = E // 128             # e partition blocks
        self.SB = SB                   # band s-sub size
        assert S % SB == 0
        self.NSS = S // SB
        self.NST = (S + 127) // 128    # s tiles of <=128 in attention
        self.STW = min(128, S)
        assert self.STW == 128 and self.NST == 2
        self.no_collective = no_collective
        self.reps = reps
        self.skip_band = skip_band
        self.skip_attn = skip_attn
        self.nmax = nmax if nmax is not None else self.CH
        self.tune = dict(ps_a=3, ps_b=4, ps_t=1, dpool=3, scan=3, qkp=18, PT=18,
                         vp=10, osc=6, oo=4, lseflat=6)
        if tune: self.tune.update(tune)

    def key(self):
        return (self.S, self.L, self.E, self.H, self.NC, self.OFF, self.SB,
                self.no_collective, self.reps,
                self.skip_band, self.skip_attn, self.nmax,
                tuple(sorted(self.tune.items())))


def build_nc(cfg: Cfg) -> bass.Bass:
    S, L, E, H, NC = cfg.S, cfg.L, cfg.E, cfg.H, cfg.NC
    CH, EB, SB, HD = cfg.CH, cfg.EB, cfg.SB, cfg.HD
    NSS = cfg.NSS
    BW = SB * CH                       # band tile free width
    NST, STW = cfg.NST, cfg.STW
    NPAIR = 2

    nc = bacc.Bacc(None)
    # ---- parameters
    xband = nc.declare_dram_parameter("xband", [E, S, CH], F16, isOutput=False)
    xattn = nc.declare_dram_parameter("xattn", [E, CH, S], F16, isOutput=False)
    xp = nc.declare_dram_parameter("xp", [E, S, CH], F16, isOutput=False)
    wband = nc.declare_dram_parameter("wband", [6, E, CH], F16, isOutput=False)
    gate_in = nc.declare_dram_parameter("gate_in", [128, BW], F16, isOutput=False)
    bqin = nc.declare_dram_parameter("bqin", [E, S], F16, isOutput=False)
    bkin = nc.declare_dram_parameter("bkin", [E, S], F16, isOutput=False)
    wq = nc.declare_dram_parameter("wq", [E, E], F16, isOutput=False)
    wk = nc.declare_dram_parameter("wk", [E, E], F16, isOutput=False)
    wv = nc.declare_dram_parameter("wv", [E, E], F16, isOutput=False)
    wo = nc.declare_dram_parameter("wo", [E, E], F16, isOutput=False)
    biasv = nc.declare_dram_parameter("biasv", [1, E], F16, isOutput=False)
    biasc = nc.declare_dram_parameter("biasc", [E, 4], F32, isOutput=False)
    ident_in = nc.declare_dram_parameter("ident_in", [128, 128], F16, isOutput=False)
    out = nc.declare_dram_parameter("out", [E, CH, S], F16, isOutput=True)

    # ---- internal DRAM
    qdram = nc.dram_tensor("qdram", [E, CH, S], F16)
    kdram = nc.dram_tensor("kdram", [E, CH, S], F16)

    with tile.TileContext(nc) as tc:
        with (
            tc.tile_pool(name="const", bufs=1) as cpool,
            tc.tile_pool(name="ps_a", bufs=cfg.tune["ps_a"], space="PSUM") as ps_a,
            tc.tile_pool(name="ps_b", bufs=cfg.tune["ps_b"], space="PSUM") as ps_b,
            tc.tile_pool(name="ps_t", bufs=cfg.tune["ps_t"], space="PSUM") as ps_t,
        ):
            # ================= constants =================
            gate = cpool.tile([128, BW], F16, name="gate")
            nc.sync.dma_start(gate[:], gate_in[:, :])
            ident = cpool.tile([128, 128], F16, name="ident")
            nc.sync.dma_start(ident[:], ident_in[:, :])

            biasv_sb = cpool.tile([1, E], F16, name="biasv_sb")
            nc.sync.dma_start(biasv_sb[:], biasv[:, :])
            biasc_sb = cpool.tile([128, 4 * EB], F32, name="biasc_sb")
            for eb in range(EB):
                nc.sync.dma_start(biasc_sb[:, 4 * eb:4 * (eb + 1)],
                                  biasc[eb * 128:(eb + 1) * 128, :])
            ones_row = cpool.tile([1, max(S, 128)], F16, name="ones_row")
            nc.vector.memset(ones_row[:], 1.0)
            ones2 = cpool.tile([2, 128], F16, name="ones2")
            nc.vector.memset(ones2[:], 1.0)

            wband_sb = []
            for kind in range(6):
                row = []
                for eb in range(EB):
                    t = cpool.tile([128, CH], F16, name=f"wband_{kind}_{eb}")
                    nc.sync.dma_start(t[:], wband[kind, eb * 128:(eb + 1) * 128, :])
                    row.append(t)
                wband_sb.append(row)

            def load_w(dram, nm):
                tiles = []
                for eb in range(EB):
                    t = cpool.tile([128, E], F16, name=f"{nm}_{eb}")
                    nc.sync.dma_start(t[:], dram[eb * 128:(eb + 1) * 128, :])
                    tiles.append(t)
                return tiles

            wq_sb = load_w(wq, "wq")
            wk_sb = load_w(wk, "wk")
            wv_sb = load_w(wv, "wv")
            wo_sb = load_w(wo, "wo")

            # host-computed boundary terms per eb
            bq_eb, bk_eb = [], []
            for eb in range(EB):
                er = slice(eb * 128, (eb + 1) * 128)
                t = cpool.tile([128, S], F16, name=f"bq_{eb}")
                nc.sync.dma_start(t[:], bqin[er, :])
                bq_eb.append(t)
                t = cpool.tile([128, S], F16, name=f"bk_{eb}")
                nc.sync.dma_start(t[:], bkin[er, :])
                bk_eb.append(t)

            def emit_band():
                with (
                    tc.tile_pool(name="bin", bufs=2) as binp,
                    tc.tile_pool(name="prod", bufs=5) as ppool,
                    tc.tile_pool(name="scan", bufs=cfg.tune["scan"]) as spool,
                    tc.tile_pool(name="asm", bufs=2) as apool,
                ):
                    for eb in range(EB):
                        er = slice(eb * 128, (eb + 1) * 128)
                        for ss in range(NSS):
                            sr = slice(ss * SB, (ss + 1) * SB)
                            xb = binp.tile([128, BW], F16, name="xb", tag="xb")
                            nc.sync.dma_start(xb[:], xband[er, sr, :])
                            xpb = binp.tile([128, BW], F16, name="xpb", tag="xpb")
                            nc.sync.dma_start(xpb[:], xp[er, sr, :])
                            x3 = xb[:].rearrange("p (s l) -> p s l", l=CH)
                            xp3 = xpb[:].rearrange("p (s l) -> p s l", l=CH)

                            def prod(kind, src3, nm, eng):
                                p = ppool.tile([128, BW], F16, name=nm,
                                               tag=f"prod_{eng}",
                                               bufs=(4 if eng == "g" else 2))
                                p3 = p[:].rearrange("p (s l) -> p s l", l=CH)
                                e = nc.gpsimd if eng == "g" else nc.vector
                                wb = wband_sb[kind][eb][:].unsqueeze(1) \
                                    .broadcast_to([128, SB, CH])
                                e.tensor_tensor(p3, src3, wb, op=ALU.mult)
                                return p

                            def scan(p, nm):
                                o = spool.tile([128, BW], F16, name=nm, tag="scan")
                                nc.vector.tensor_tensor_scan(
                                    o[:], gate[:], p[:], 0.0,
                                    op0=ALU.mult, op1=ALU.add)
                                return o

                            def half(qk, kf, ks, kp_, dram, store_eng):
                                # kf: fwd kind, ks: bwd kind, kp_: partner
                                peng = "v" if qk == "q" else "g"
                                pf = prod(kf, x3, "pf", "g")
                                ps_ = prod(ks, x3, "ps", "g")
                                pp = prod(kp_, xp3, "pp", peng)
                                # combined scan of (shift(pf) + pp - ps); the
                                # fwd shift is applied while combining
                                c1 = apool.tile([128, BW], F16, name=f"c1{qk}",
                                                tag="ts")
                                c13 = c1[:].rearrange("p (s l) -> p s l", l=CH)
                                pf3 = pf[:].rearrange("p (s l) -> p s l", l=CH)
                                pp3 = pp[:].rearrange("p (s l) -> p s l", l=CH)
                                nc.vector.tensor_tensor(
                                    c13[:, :, 1:CH], pf3[:, :, 0:CH - 1],
                                    pp3[:, :, 1:CH], op=ALU.add)
                                nc.vector.tensor_copy(c13[:, :, 0:1],
                                                      pp3[:, :, 0:1])
                                c2 = apool.tile([128, BW], F16, name=f"c2{qk}",
                                                tag="t1")
                                nc.vector.tensor_tensor(c2[:], c1[:], ps_[:],
                                                        op=ALU.subtract)
                                I = scan(c2, "I")
                                t2 = apool.tile([128, BW], F16, name=f"t2{qk}",
                                                tag="ts")
                                nc.vector.tensor_tensor(t2[:], xb[:], I[:],
                                                        op=ALU.add)
                                # free-dim permute (s,l)->(l,s) on Scalar
                                o2 = apool.tile([128, BW], F16, name=f"o2{qk}",
                                                tag="o2")
                                nc.scalar.copy(
                                    o2[:].rearrange("p (l s) -> p l s", s=SB),
                                    t2[:].rearrange("p (s l) -> p l s", l=CH))
                                store_eng.dma_start(
                                    dram[er, 0:CH, sr],
                                    o2[:].rearrange("p (l s) -> p l s", s=SB))

                            half("q", 0, 2, 4, qdram, nc.scalar)
                            half("k", 1, 3, 5, kdram, nc.sync)


            # ================= B-terms =================
            def emit_b():
                Bqp, Bkp = [], []  # fp16 [128, S] per fm, proj-space
                for qk, w_sb, B_eb, bj, dst in (
                        ("q", wq_sb, bq_eb, 0, Bqp),
                        ("k", wk_sb, bk_eb, 1, Bkp)):
                    for fm in range(EB):
                        fr = slice(fm * 128, (fm + 1) * 128)
                        acc = ps_a.tile([128, S], F32, name=f"psB{qk}{fm}",
                                        tag="ps_mm")
                        for eb in range(EB):
                            nc.tensor.matmul(acc[:], w_sb[eb][:, fr],
                                             B_eb[eb][:],
                                             start=(eb == 0), stop=(eb == EB - 1))
                        o = cpool.tile([128, S], F16, name=f"B{qk}p_{fm}")
                        nc.vector.tensor_scalar_add(
                            o[:], acc[:],
                            biasc_sb[:, 4 * fm + bj:4 * fm + bj + 1])
                        dst.append(o)
                return Bqp, Bkp

            def attn_stage1(n0, qt2, kt2, xt2, Bqp, Bkp, apool):
                """proj + v-proj + pass1 softmax stats + lse -> state dict."""
                T = cfg.tune
                NP = NPAIR

                def proj(w_sb, src2, Bp, nm):
                    outt = []
                    for fm in range(EB):
                        fr = slice(fm * 128, (fm + 1) * 128)
                        acc = ps_a.tile([128, NP * S], F32, name=f"ps{nm}{fm}",
                                        tag="ps_mm")
                        for eb in range(EB):
                            nc.tensor.matmul(acc[:], w_sb[eb][:, fr],
                                             src2[eb][:],
                                             start=(eb == 0), stop=(eb == EB - 1))
                        o = apool.tile([128, NP * S], F16, name=f"{nm}_{fm}",
                                       tag="qkp", bufs=T["qkp"])
                        for j in range(NP):
                            js = slice(j * S, (j + 1) * S)
                            nc.vector.tensor_tensor(o[:, js], acc[:, js],
                                                    Bp[fm][:], op=ALU.add)
                        outt.append(o)
                    return outt

                qp = proj(wq_sb, qt2, Bqp, "qp")
                kp = proj(wk_sb, kt2, Bkp, "kp")

                def hv(p, h, j):
                    return p[h][:, j * S:(j + 1) * S]

                vp = [[None] * NST for _ in range(NP)]
                for j in range(NP):
                    for st in range(NST):
                        scols = slice(j * S + st * 128, j * S + st * 128 + STW)
                        acc = ps_a.tile([STW, E], F32, name=f"psv{j}{st}",
                                        tag="ps_mm")
                        for eb in range(EB):
                            nc.tensor.matmul(acc[:], xt2[eb][:, scols],
                                             wv_sb[eb][:],
                                             start=(eb == 0), stop=False)
                        nc.tensor.matmul(acc[:], ones_row[:1, :STW],
                                         biasv_sb[:1, :], start=False, stop=True)
                        o = apool.tile([STW, E], F16, name=f"vp{j}{st}",
                                       tag="vp", bufs=T["vp"])
                        nc.vector.tensor_copy(o[:], acc[:])
                        vp[j][st] = o

                # pass 1: (s,t) scores -> -max, den (both j in one den tile)
                nmax_c = []
                den_pair = apool.tile([STW, NP * 2 * H], F32, name="denp",
                                      tag="denp", bufs=3)
                escr = apool.tile([STW, S], F16, name="escr", tag="escr", bufs=2)
                for j in range(NP):
                    nm_ = apool.tile([STW, 2 * H], F32, name=f"nmaxc{j}",
                                     tag="nmaxc", bufs=4)
                    nmax_c.append(nm_)
                    for st in range(NST):
                        scols = slice(st * 128, st * 128 + STW)
                        for h in range(H):
                            c = h * NST + st
                            accs = ps_b.tile([STW, S], F32, name=f"ps1{j}{st}{h}",
                                             tag="ps_sc")
                            nc.tensor.matmul(accs[:], hv(qp, h, j)[:, scols],
                                             hv(kp, h, j), start=True, stop=True)
                            nc.vector.tensor_reduce(
                                nm_[:, c:c + 1], accs[:], axis=AX.X,
                                op=ALU.max, negate=True)
                            nc.scalar.activation(
                                escr[:], accs[:], ACTF.Exp,
                                bias=nm_[:, c:c + 1], scale=1.0,
                                accum_out=den_pair[:, j * 2 * H + c:j * 2 * H + c + 1])
                # ONE Ln for the whole pair
                ln_pair = apool.tile([STW, NP * 2 * H], F32, name="lnp",
                                     tag="lnp", bufs=3)
                nc.scalar.activation(ln_pair[:], den_pair[:], ACTF.Ln)
                lseflat = []
                for j in range(NP):
                    lse32 = apool.tile([STW, 2 * H], F32, name=f"lse32{j}",
                                       tag="lse32", bufs=4)
                    nc.vector.tensor_tensor(
                        lse32[:], nmax_c[j][:],
                        ln_pair[:, j * 2 * H:(j + 1) * 2 * H],
                        op=ALU.subtract)  # -(max+ln den)
                    pk = apool.tile([STW, 4 * H], F16, name=f"lsepack{j}",
                                    tag="lsepack", bufs=4)
                    nc.vector.tensor_copy(pk[:, 0:2 * H], lse32[:])
                    resid = apool.tile([STW, 2 * H], F32, name=f"resid{j}",
                                       tag="resid", bufs=4)
                    nc.vector.tensor_tensor(resid[:], lse32[:], pk[:, 0:2 * H],
                                            op=ALU.subtract)
                    nc.vector.tensor_copy(pk[:, 2 * H:4 * H], resid[:])
                    lf = apool.tile([1, STW * 4 * H], F16, name=f"lseflat{j}",
                                    tag="lseflat", bufs=T["lseflat"])
                    nc.sync.dma_start(
                        lf[:].rearrange("o (s r) -> o s r", r=4 * H), pk[:])
                    lseflat.append(lf)
                return dict(n0=n0, qp=qp, kp=kp, vp=vp, lseflat=lseflat, hv=hv)

            def attn_stage2(stt, apool):
                """pass2 + attn@V + out-projection for a stage1'd pair."""
                T = cfg.tune
                NP = NPAIR
                n0, qp, kp, vp, lseflat, hv = (stt["n0"], stt["qp"], stt["kp"],
                                               stt["vp"], stt["lseflat"],
                                               stt["hv"])
                PT = [[[None] * NST for _ in range(H)] for _ in range(NP)]
                for j in range(NP):
                    lse_rs = lseflat[j][:].rearrange("o (s r) -> o r s",
                                                     r=4 * H)
                    for grp in range(2):
                        accs2 = []
                        for h2 in range(2):
                            h = grp * 2 + h2
                            for tt in range(NST):
                                tcols = slice(tt * 128, tt * 128 + STW)
                                acc = ps_b.tile([STW, S], F32,
                                                name=f"ps2{j}{h}{tt}",
                                                tag="ps_sc")
                                nc.tensor.matmul(acc[:], hv(kp, h, j)[:, tcols],
                                                 hv(qp, h, j),
                                                 start=True, stop=False)
                                accs2.append((acc, h, tt))
                        for acc, h, tt in accs2:
                            for part in range(2):
                                r0 = part * 2 * H + h * NST
                                nc.tensor.matmul(
                                    acc[:], ones_row[:1, :STW],
                                    lse_rs[:, r0:r0 + NST, :],
                                    start=False, stop=(part == 1))
                            p = apool.tile([STW, S], F16, name=f"PT{j}{h}{tt}",
                                           tag="PT", bufs=T["PT"])
                            nc.scalar.activation(p[:], acc[:], ACTF.Exp)
                            PT[j][h][tt] = p

                osc = []
                for h in range(H):
                    hr = slice(h * HD, (h + 1) * HD)
                    acc = ps_t.tile([HD, NP * S], F32, name=f"pso{h}",
                                    tag="ps_oo")
                    for j in range(NP):
                        js = slice(j * S, (j + 1) * S)
                        for tt in range(NST):
                            nc.tensor.matmul(acc[:, js], vp[j][tt][:, hr],
                                             PT[j][h][tt][:],
                                             start=(tt == 0), stop=(tt == NST - 1))
                    o = apool.tile([HD, NP * S], F16, name=f"osc{h}", tag="osc",
                                   bufs=T["osc"])
                    nc.vector.tensor_copy(o[:], acc[:])
                    osc.append(o)

                for gm in range(EB):
                    gr = slice(gm * 128, (gm + 1) * 128)
                    acc = ps_a.tile([128, NP * S], F32, name=f"psout{gm}",
                                    tag="ps_mm")
                    for fm in range(EB):
                        nc.tensor.matmul(acc[:], wo_sb[fm][:, gr], osc[fm][:],
                                         start=(fm == 0), stop=(fm == EB - 1))
                    o = apool.tile([128, NP * S], F16, name=f"oo{gm}", tag="oo",
                                   bufs=T["oo"])
                    nc.vector.tensor_scalar_add(
                        o[:], acc[:], biasc_sb[:, 4 * gm + 3:4 * gm + 4])
                    nc.scalar.dma_start(
                        out[gr, n0:n0 + NP, :],
                        o[:].rearrange("p (j s) -> p j s", j=NP))

            def emit_attn_all(Bqp, Bkp):
                with (
                    tc.tile_pool(name="dpool", bufs=cfg.tune["dpool"]) as dpool,
                    tc.tile_pool(name="attn", bufs=2) as apool,
                ):
                    NMAX = cfg.nmax if not cfg.skip_attn else 0
                    assert NMAX % NPAIR == 0
                    prev = None
                    for n0 in range(0, NMAX, NPAIR):
                        qt2, kt2, xt2 = [], [], []
                        nsl = slice(n0, n0 + NPAIR)
                        for eb in range(EB):
                            er = slice(eb * 128, (eb + 1) * 128)
                            t = dpool.tile([128, NPAIR * S], F16, name=f"qt{eb}",
                                           tag=f"qt{eb}")
                            nc.sync.dma_start(t[:], qdram[er, nsl, :])
                            qt2.append(t)
                            t = dpool.tile([128, NPAIR * S], F16, name=f"kt{eb}",
                                           tag=f"kt{eb}")
                            nc.sync.dma_start(t[:], kdram[er, nsl, :])
                            kt2.append(t)
                            t = dpool.tile([128, NPAIR * S], F16, name=f"xt{eb}",
                                           tag=f"xt{eb}")
                            nc.sync.dma_start(t[:], xattn[er, nsl, :])
                            xt2.append(t)
                        cur = attn_stage1(n0, qt2, kt2, xt2, Bqp, Bkp, apool)
                        if prev is not None:
                            attn_stage2(prev, apool)
                        prev = cur
                    if prev is not None:
                        attn_stage2(prev, apool)

            for _rep in range(cfg.reps):
                if not cfg.skip_band:
                    emit_band()
                Bqp, Bkp = emit_b()
                emit_attn_all(Bqp, Bkp)

    nc.finalize()
    return nc


# ============================================================
# host side
# ============================================================

def prep_inputs(cfg: Cfg, x, a, b, c, d, in_proj_w, in_proj_b, out_w, out_b):
    S, L, E, NC, CH, OFF = cfg.S, cfg.L, cfg.E, cfg.NC, cfg.CH, cfg.OFF
    f32, f16 = np.float32, np.float16
    x = np.asarray(x, f32)
    xg = np.ascontiguousarray(x.transpose(2, 0, 1))     # (E, S, L)
    hd = cfg.HD
    scl = 1.0 / math.sqrt(hd)
    wq = np.ascontiguousarray(in_proj_w[:E].T * scl).astype(f16)
    wk = np.ascontiguousarray(in_proj_w[E:2 * E].T).astype(f16)
    wv = np.ascontiguousarray(in_proj_w[2 * E:].T).astype(f16)
    wo = np.ascontiguousarray(out_w.T).astype(f16)
    bq = in_proj_b[:E] * scl
    bk = in_proj_b[E:2 * E]
    bv = in_proj_b[2 * E:]
    bo = out_b
    biasv = np.asarray(bv, f16).reshape(1, E)
    # last-column fwd weights (per core below)
    biasc = np.ascontiguousarray(
        np.stack([bq, bk, bv, bo]).astype(f32).T)       # (E, 4)
    ident = np.eye(128, dtype=f16)

    gate = np.ones((128, cfg.SB * CH), f16)
    gate[:, ::CH] = 0.0

    # boundary chunk totals: T[kind][j][e,s] = sum_{l in chunk j} x[s,l,e]*w[l,e]
    xr = x.reshape(S, NC, CH, E)
    Tt = {}
    for nmw, w in (("a", a), ("b", b), ("c", c), ("d", d)):
        Tt[nmw] = np.einsum("sjle,jle->jes", xr,
                            np.asarray(w, f32).reshape(NC, CH, E),
                            optimize=True)

    in_maps = []
    for k in range(NC):
        chk = slice(CH * k, CH * (k + 1))
        xbandc = np.ascontiguousarray(xg[:, :, chk]).astype(f16)
        xattnc = np.ascontiguousarray(
            xg[:, :, chk].transpose(0, 2, 1)).astype(f16)
        if k >= OFF:
            pf = slice(CH * (k - OFF), CH * (k - OFF + 1))
            xpc = np.ascontiguousarray(xg[:, :, pf]).astype(f16)
            w1 = -a[pf].astype(f32)
            w2 = -b[pf].astype(f32)
        else:
            st = CH * (k + OFF) - 1
            xpc = np.zeros((E, S, CH), f16)
            xpc[:, :, 1:] = xg[:, :, st + 1:st + CH]
            w1 = np.zeros((CH, E), f32)
            w1[1:] = c[st + 1:st + CH]
            w2 = np.zeros((CH, E), f32)
            w2[1:] = d[st + 1:st + CH]
        wbandc = np.ascontiguousarray(
            np.stack([a[chk], b[chk], c[chk], d[chk], w1, w2])
            .transpose(0, 2, 1)).astype(f16)            # (6, E, CH)
        jA = slice(max(0, k - OFF), k)
        jC = slice(k, min(k + OFF - 1, NC - 1) + 1)
        bqc = (Tt["a"][jA].sum(0) + Tt["c"][jC].sum(0)).astype(f16)
        bkc = (Tt["b"][jA].sum(0) + Tt["d"][jC].sum(0)).astype(f16)
        in_maps.append(dict(
            xband=xbandc, xattn=xattnc, xp=xpc,
            wband=wbandc, gate_in=gate, bqin=bqc, bkin=bkc,
            wq=wq, wk=wk, wv=wv, wo=wo, biasv=biasv, biasc=biasc,
            ident_in=ident,
        ))
    return in_maps


_CACHE = {}


def run(cfg: Cfg, inputs, core_ids=None, **kw):
    key = cfg.key()
    if key not in _CACHE:
        _CACHE[key] = build_nc(cfg)
    nc = _CACHE[key]
    in_maps = prep_inputs(
        cfg, inputs["x"], inputs["a"], inputs["b"], inputs["c"], inputs["d"],
        inputs["in_proj_w"], inputs["in_proj_b"], inputs["out_w"], inputs["out_b"])
    res = run_bass_kernel_spmd(nc, in_maps, core_ids or list(range(cfg.NC)), **kw)
    S, L, E, CH = cfg.S, cfg.L, cfg.E, cfg.CH
    full = np.empty((S, L, E), np.float32)
    for k in range(cfg.NC):
        # out is (E, CH, S) fp16
        full[:, CH * k:CH * (k + 1), :] = \
            res.results[k]["out"].astype(np.float32).transpose(2, 1, 0)
    return full, res


def kernel(**inputs) -> np.ndarray:
    assert int(inputs["n1"]) == 256 and int(inputs["n2"]) == 256
    cfg = Cfg()
    out, _ = run(cfg, inputs)
    return out
